# revision 49
# baseline (speedup 1.0000x reference)
"""GAT (2-layer graph attention network) on 8 Trainium2 NeuronCores.

v2 strategy (1D node partition; rank-1 max factorization of the scores):
  exp(leaky_relu(s1_i + s2_j)) = max(exp(s1+s2), exp(a(s1+s2)))
and dividing column i by exp(a*s1_i) (softmax-invariant per i) gives the
softmax-equivalent unnormalized weight
  z[j,i] = Q_j * max(G_i, r_j) * M[j,i]
with G_i = exp((1-a)s1_i), r_j = exp(-(1-a)s2_j), Q_j = exp(s2_j) and a
multiplicative {0,1} adjacency mask M. Per score tile this is ONE DVE
tensor_scalar (op0=max with r, op1=mult with Q; 4x perf mode) plus a
group-batched tensor_tensor mask multiply (2x mode; ~1/4 of groups run on
the otherwise-idle GPSIMD engine) — no per-element exp, no custom DVE
score op. Exps only run on O(N) projection vectors (Act engine).

  - Each core owns R = N/8 rows; computes local Wh = X_loc @ W1 plus the 8
    fused score projections (W1 @ a-halves folded host-side), AllGathers
    [Wh | proj] in FOUR per-row-tile chunks so the gather pipeline overlaps
    local compute and the first attention groups.
  - The mask is stored rt-major so attention groups are (rt, c-quad) tiles
    and each gather chunk unblocks a full stripe of groups.
  - Aggregation numerator and denominator are PE matmuls accumulating over
    j-tiles (denominator lhsT = ones). Score production is software-
    pipelined 5 groups ahead of PE consumption (keeps PE at max p-state).
  - Layer 2 identical with a half-split gather; its denominator rides as
    column 64 of the gathered stationary operand.

Numerics: matmuls bf16 (f32 PSUM), mask exact {0,1}, z products bf16.
"""

import math
from contextlib import ExitStack
from dataclasses import dataclass

import numpy as np
import ml_dtypes

import concourse.bass as bass
import concourse.mybir as mybir
import concourse.tile as tile
from concourse import bacc
from concourse.bass_utils import run_bass_kernel_spmd

BF16 = ml_dtypes.bfloat16
ALPHA = 0.2

# --------------------------------------------------------------------------
# Custom fused DVE op for elu (registered into concourse.dve_ops at import)
# --------------------------------------------------------------------------

import concourse.dve_ops as dve_ops
from concourse.dve_spec import (
    Spec, Src0, Src1, C0, Zero, lower, select, _has_src1,
)
from concourse.dve_uop import DveOpSpec


def _make_elu_spec():
    # out = in0 > 0 ? in0 : in1 - s0   (elu with in1 = exp(in0), s0 = 1.0)
    def _elu_ref(in0, in1, s0, s1, imm2):
        x = in0.astype(np.float32)
        return np.where(x > 0, x, in1.astype(np.float32) - s0)

    return Spec(body=select(Src0 > Zero, Src0, Src1 - C0), reference=_elu_ref)


def _register(name, spec):
    if name in dve_ops._SUB_OPCODE_FOR_NAME:
        for op in dve_ops.OPS:
            if op.name == name:
                return op
    row = max(dve_ops._SUB_OPCODE_FOR_NAME.values()) + 1
    assert row < 0x20
    shas = {}
    for ver in ("v3", "v4"):
        uops = lower(spec, ver=ver)
        shas[ver] = DveOpSpec(
            name=name, opcode=row, uops=uops, rd1_en=_has_src1(spec)
        ).sha(ver)
    op = dve_ops.DveOp(name, spec, subdim=False, uops_sha=shas)
    dve_ops.OPS.append(op)
    dve_ops.CUSTOM_DVE_SPECS[name] = spec
    dve_ops._SUB_OPCODE_FOR_NAME[name] = row
    return op


ELU_SEL = _register("ELU_SEL_GAT", _make_elu_spec())


# --------------------------------------------------------------------------
# Kernel configuration
# --------------------------------------------------------------------------

@dataclass(frozen=True)
class Cfg:
    N: int = 4096      # nodes
    C: int = 512       # input feature dim
    H: int = 128       # hidden per head (must be 128)
    HEADS: int = 4
    F2: int = 64       # output dim
    CORES: int = 8
    GRP: int = 4       # j-tiles per batched mask multiply

    @property
    def R(self): return self.N // self.CORES          # rows per core
    @property
    def JT(self): return self.N // 128                # j tiles
    @property
    def CT(self): return self.C // 128                # input-feature tiles
    @property
    def HH(self): return self.HEADS * self.H          # layer-1 out features
    @property
    def CT2(self): return self.HH // 128              # layer-2 contraction tiles
    @property
    def RT(self): return self.R // 128                # local row tiles
    @property
    def S8(self): return 2 * self.HEADS               # score projections per node
    @property
    def PAY(self): return self.F2 + 6                 # l2 gather cols (64|1|pad|s1s2f32)
    @property
    def QWID(self): return self.HH + 2 * self.S8      # l1 per-rt gather cols (528)
    @property
    def NCQ(self): return self.CORES // 4             # c-quad groups per rt (2)
    @property
    def NG(self): return self.JT // self.GRP


FULL = Cfg()


# --------------------------------------------------------------------------
# Device program
# --------------------------------------------------------------------------

def build_gat_nc(cfg: Cfg, collective: bool = True, iters: int = 1,
                 loop_iters: int = 0, phases: str = "full"):
    dt = mybir.dt.bfloat16
    f32 = mybir.dt.float32
    add = mybir.AluOpType.add
    mult = mybir.AluOpType.mult
    maxop = mybir.AluOpType.max
    bypass = mybir.AluOpType.bypass
    Exp = mybir.ActivationFunctionType.Exp

    N, C, HEADS, F2, R = cfg.N, cfg.C, cfg.HEADS, cfg.F2, cfg.R
    JT, CT, HH, CT2, RT = cfg.JT, cfg.CT, cfg.HH, cfg.CT2, cfg.RT
    S8, PAY, GRP, NG = cfg.S8, cfg.PAY, cfg.GRP, cfg.NG
    QWID, NCQ = cfg.QWID, cfg.NCQ
    F2p = F2 + 2
    BETA = 1.0 - ALPHA         # 0.8
    PIPE = 5

    nc = bacc.Bacc(
        "TRN2", target_bir_lowering=False, debug=False, num_devices=cfg.CORES
    )

    # ---- DRAM I/O -------------------------------------------------------
    # xtloc is rt-major: [128, rt*C + ct*128 + j]
    # mb is rt-major: [128, (rt*CORES + c)*R + i] for adjacency tile c*RT+rt
    xtl_d = nc.dram_tensor("xtloc", [128, CT * R], dt, kind="ExternalInput").ap()
    mb_d = nc.dram_tensor("mb", [128, JT * R], dt, kind="ExternalInput").ap()
    w1c_d = nc.dram_tensor("w1c", [128, CT * HH], dt, kind="ExternalInput").ap()
    ws1_d = nc.dram_tensor("ws1", [128, CT * S8], dt, kind="ExternalInput").ap()
    w2a_d = nc.dram_tensor("w2a", [128, CT2 * F2p], dt, kind="ExternalInput").ap()
    id_d = nc.dram_tensor("ident", [128, 128], dt, kind="ExternalInput").ap()
    idf_d = nc.dram_tensor("identf", [128, 128], f32, kind="ExternalInput").ap()
    out_d = nc.dram_tensor("out", [R, F2], f32, kind="ExternalOutput").ap()

    with tile.TileContext(nc) as tc, ExitStack() as ctx:
        const = ctx.enter_context(tc.tile_pool(name="const", bufs=1))
        work = ctx.enter_context(tc.tile_pool(name="work", bufs=3))
        wz = ctx.enter_context(tc.tile_pool(name="wz", bufs=7))
        psb = ctx.enter_context(tc.tile_pool(name="psb", bufs=3, space="PSUM"))
        pss = ctx.enter_context(tc.tile_pool(name="pss", bufs=2, space="PSUM"))
        psd = ctx.enter_context(tc.tile_pool(name="psd", bufs=2, space="PSUM"))
        ps2 = ctx.enter_context(tc.tile_pool(name="ps2", bufs=1, space="PSUM"))
        dram = ctx.enter_context(tc.tile_pool(name="dram", bufs=1, space="DRAM"))

        # per-rt l1 gather payload: [whl | prl]
        whsend = [dram.tile([128, QWID], dt, name=f"whsend{i}")
                  for i in range(RT)]
        # l2 gather halves: rt 0-1, rt 2-3
        gsend = [dram.tile([128, 2 * PAY], dt, name=f"gsend{i}")
                 for i in range(2)]
        if cfg.CORES > 4:
            whfull = [
                nc.dram_tensor(f"whfull{i}_sh", [cfg.CORES * 128, QWID], dt,
                               addr_space="Shared").ap()
                for i in range(RT)
            ]
            gfull = [
                nc.dram_tensor(f"gfull{i}_sh", [cfg.CORES * 128, 2 * PAY], dt,
                               addr_space="Shared").ap()
                for i in range(2)
            ]
        else:
            whfull = [dram.tile([cfg.CORES * 128, QWID], dt,
                                name=f"whfull{i}") for i in range(RT)]
            gfull = [dram.tile([cfg.CORES * 128, 2 * PAY], dt,
                               name=f"gfull{i}") for i in range(2)]

        import contextlib
        loop_cm = (tc.For_i(0, loop_iters, 1) if loop_iters
                   else contextlib.nullcontext())
        with loop_cm:
          for _it in range(iters):
            # ---- constant loads ------------------------------------------
            mb_sb = const.tile([128, JT * R], dt)
            _hm = JT * R // 4          # one rt stripe of the mask
            xtl_sb = const.tile([128, CT * R], dt)
            for rt in range(RT):
                nc.sync.dma_start(out=xtl_sb[:, rt * C: (rt + 1) * C],
                                  in_=xtl_d[:, rt * C: (rt + 1) * C])
            w1c_sb = const.tile([128, CT * HH], dt)
            for ct in range(CT):
                nc.sync.dma_start(out=w1c_sb[:, ct * HH: (ct + 1) * HH],
                                  in_=w1c_d[:, ct * HH: (ct + 1) * HH])
            ws1_sb = const.tile([128, CT * S8], dt)
            nc.sync.dma_start(out=ws1_sb, in_=ws1_d)
            ident_sb = const.tile([128, 128], dt)
            nc.sync.dma_start(out=ident_sb, in_=id_d)
            _he = JT * R // 8          # half a stripe (one c-quad)
            w2a_sb = const.tile([128, CT2 * F2p], dt)
            nc.sync.dma_start(out=w2a_sb, in_=w2a_d)
            identf_sb = const.tile([128, 128], f32)
            nc.sync.dma_start(out=identf_sb, in_=idf_d)

            ones_col = const.tile([128, 1], dt)
            nc.vector.memset(ones_col, 1.0)

            # ---- P1: local Wh + projections, per-rt gather dispatch ------
            # wh_sb/prj/rqr are all t-major with t = c*RT + rt
            wh_sb = const.tile([128, JT * HH], dt)
            prj_sb = const.tile([128, JT * 2 * S8], dt)
            rqr_sb = const.tile([128, JT * S8], f32)   # exp(-BETA*proj)
            rqq_sb = const.tile([128, JT * S8], f32)   # exp(proj)
            wh3 = wh_sb[:, :].rearrange("p (c r) -> p c r", c=cfg.CORES)
            prj3 = prj_sb[:, :].rearrange("p (c q) -> p c q", c=cfg.CORES)
            prjf3 = prj_sb[:, :].bitcast(f32).rearrange(
                "p (c q) -> p c q", c=cfg.CORES)
            rqr3 = rqr_sb[:, :].rearrange("p (c q) -> p c q", c=cfg.CORES)
            rqq3 = rqq_sb[:, :].rearrange("p (c q) -> p c q", c=cfg.CORES)
            # per-head G rows at partition 0 (partition_broadcast needs it)
            g1row = [const.tile([1, R], dt, name=f"g1row{h}")
                     for h in range(HEADS)]
            wst = [const.tile([128, QWID], dt, name=f"wst{i}")
                   for i in range(RT)]
            for rt in range(RT):
                stg = wst[rt]
                pA = psb.tile([128, HH], f32, tag="big", name=f"pA{rt}")
                pP = pss.tile([128, S8], f32, tag="sm", name=f"pP{rt}")
                for ct in range(CT):
                    xsl = xtl_sb[:, rt * C + ct * 128: rt * C + (ct + 1) * 128]
                    nc.tensor.matmul(
                        out=pA, lhsT=xsl,
                        rhs=w1c_sb[:, ct * HH: (ct + 1) * HH],
                        start=(ct == 0), stop=(ct == CT - 1),
                    )
                for ct in range(CT):
                    xsl = xtl_sb[:, rt * C + ct * 128: rt * C + (ct + 1) * 128]
                    nc.tensor.matmul(
                        out=pP, lhsT=xsl,
                        rhs=ws1_sb[:, ct * S8: (ct + 1) * S8],
                        start=(ct == 0), stop=(ct == CT - 1),
                    )
                nc.scalar.copy(out=stg[:, 0: HH], in_=pA)
                if rt == 0:
                    # Act queue is in-order: issues only after the whl copy
                    # unblocks, keeping early DMA engines free for the
                    # critical path
                    for _e in range(2, 4):
                        nc.scalar.dma_start(
                            out=mb_sb[:, _e * _he: (_e + 1) * _he],
                            in_=mb_d[:, _e * _he: (_e + 1) * _he])
                prlf = stg[:, HH: QWID].bitcast(f32)
                nc.vector.tensor_copy(out=prlf, in_=pP)
                # local s1 rows -> per-head G rows at partition 0
                for h in range(HEADS):
                    pt = pss.tile([1, 128], f32, tag="sm", name=f"pt{rt}_{h}")
                    nc.tensor.transpose(
                        out=pt, in_=prlf[:, 2 * h: 2 * h + 1],
                        identity=identf_sb)
                    nc.scalar.activation(
                        out=g1row[h][0:1, rt * 128: (rt + 1) * 128],
                        in_=pt, func=Exp, scale=BETA)
                nc.sync.dma_start(out=whsend[rt][:, :], in_=stg)
                if collective:
                    nc.gpsimd.collective_compute(
                        "AllGather", bypass,
                        replica_groups=[list(range(cfg.CORES))],
                        ins=[whsend[rt].opt()], outs=[whfull[rt].opt()],
                    )
                else:
                    nc.sync.dma_start(
                        out=whfull[rt].rearrange("(c p) q -> c p q", p=128),
                        in_=whsend[rt][:, :].unsqueeze(0).broadcast_to(
                            (cfg.CORES, 128, QWID)))
                if rt == 0:
                    for _e in range(2):
                        nc.sync.dma_start(
                            out=mb_sb[:, _e * _he: (_e + 1) * _he],
                            in_=mb_d[:, _e * _he: (_e + 1) * _he])
                # land this rt's gathered chunk + its r/Q exps immediately,
                # in c-halves so the first c-quad group unblocks early
                full3 = whfull[rt].rearrange("(c p) q -> p c q", p=128)
                sl = slice(rt * S8, (rt + 1) * S8)
                slp = slice(rt * 2 * S8, (rt + 1) * 2 * S8)
                for ch in range(2):
                    cs = slice(ch * 4, ch * 4 + 4)
                    # proj first: it gates the rqr/rqq exps
                    nc.scalar.dma_start(
                        out=prj3[:, cs, slp], in_=full3[:, cs, HH: QWID])
                    nc.scalar.dma_start(
                        out=wh3[:, cs, rt * HH: (rt + 1) * HH],
                        in_=full3[:, cs, 0: HH])
                    nc.scalar.activation(
                        out=rqr3[:, cs, sl], in_=prjf3[:, cs, sl],
                        func=Exp, scale=-BETA)
                    nc.scalar.activation(
                        out=rqq3[:, cs, sl], in_=prjf3[:, cs, sl],
                        func=Exp, scale=1.0)
                if rt == 2:
                    # defer the mask tail so its DMA traffic overlaps l1
                    for _e in range(4, 8):
                        nc.scalar.dma_start(
                            out=mb_sb[:, _e * _he: (_e + 1) * _he],
                            in_=mb_d[:, _e * _he: (_e + 1) * _he])

            gb = []
            for h in range(HEADS):
                g_t = const.tile([128, R], dt, name=f"g_t{h}")
                nc.gpsimd.partition_broadcast(
                    out_ap=g_t[:, :], in_ap=g1row[h][0:1, :])
                gb.append(g_t)

            if phases == "wh":
                for rt in range(RT):
                    nc.sync.dma_start(out=out_d[rt * 128:(rt + 1) * 128, :],
                                      in_=identf_sb[:, 0:F2])
                continue

            # ---- P4: layer-1 attention + aggregation ---------------------
            # group = (rt, c-quad): tiles t = (4*cq+ci)*RT + rt, ci 0..3
            mb5 = mb_sb[:, :].rearrange("p (r c i) -> p r c i",
                                        r=RT, c=cfg.CORES)
            hloc_sb = const.tile([128, CT2 * R], dt)   # h_local^T feature-major
            # head-pair stripes, rt-major within a pair: each gather chunk
            # feeds a stripe of groups before the next chunk is needed, and
            # only two heads' psums are ever live at once
            slots = [(h, rt, cq) for hp in range(HEADS // 2)
                     for rt in range(RT) for h in (2 * hp, 2 * hp + 1)
                     for cq in range(NCQ)]
            store = {}
            psums = {}
            head_gi = {h: 0 for h in range(HEADS)}
            gi_pool = 0
            # layer-2 projections accumulate per head as soon as its ELU is
            # out; lives in the ps2 bank which psum2 reuses afterwards
            pWall = ps2.tile([128, RT * F2p], f32, tag="big2",
                             name="pWall")

            def tiles_of(rt, cq):
                return [(4 * cq + ci) * RT + rt for ci in range(GRP)]

            def issue_scores(s, slot_list, g2=False):
                nonlocal gi_pool
                if g2:
                    rt, cq = slot_list[s]
                    h = None
                else:
                    h, rt, cq = slot_list[s]
                tmp4 = wz.tile([128, GRP * R], dt, tag="tmp", name=f"tmp{g2}{s}")
                yg = wz.tile([128, GRP * R], dt, tag="yg", name=f"yg{g2}{s}")
                tiles = tiles_of(rt, cq)
                for k, t in enumerate(tiles):
                    if g2:
                        s1ap = rq2r_sb[:, t: t + 1]
                        s2ap = rq2q_sb[:, t: t + 1]
                        g_in = g2b
                    else:
                        s1ap = rqr_sb[:, t * S8 + 2 * h + 1: t * S8 + 2 * h + 2]
                        s2ap = rqq_sb[:, t * S8 + 2 * h + 1: t * S8 + 2 * h + 2]
                        g_in = gb[h]
                    nc.vector.tensor_scalar(
                        out=tmp4[:, k * R: (k + 1) * R], in0=g_in,
                        scalar1=s1ap, scalar2=s2ap,
                        op0=maxop, op1=mult,
                    )
                tmp4r = tmp4[:, :].rearrange("p (a b i) -> p a b i", a=1, b=GRP)
                ygr = yg[:, :].rearrange("p (a b i) -> p a b i", a=1, b=GRP)
                mbs = mb5[:, rt: rt + 1, 4 * cq: 4 * cq + GRP, :]
                if g2:
                    use_pool = s in (1, 3)
                else:
                    use_pool = gi_pool % 4 == 1 and s < len(slot_list) - 3
                eng = nc.gpsimd if use_pool else nc.vector
                eng.tensor_tensor(out=ygr, in0=tmp4r, in1=mbs, op=mult)
                gi_pool += 1
                store[(g2, s)] = (tiles, yg)

            def issue_matmuls(s):
                h, rt, cq = slots[s]
                gi = head_gi[h]
                head_gi[h] += 1
                tiles, yg = store.pop((False, s))
                if gi == 0:
                    psums[h] = (psb.tile([128, R], f32, tag="big",
                                         name=f"ph{h}"),
                                psd.tile([1, R], f32, tag="den",
                                         name=f"pd{h}"))
                psum_h, psum_d = psums[h]
                for k, t in enumerate(tiles):
                    yt = yg[:, k * R: (k + 1) * R]
                    nc.tensor.matmul(
                        out=psum_h,
                        lhsT=wh_sb[:, t * HH + h * 128: t * HH + (h + 1) * 128],
                        rhs=yt,
                        start=(gi == 0 and k == 0),
                        stop=(gi == NG - 1 and k == GRP - 1),
                    )
                    nc.tensor.matmul(
                        out=psum_d, lhsT=ones_col, rhs=yt,
                        start=(gi == 0 and k == 0),
                        stop=(gi == NG - 1 and k == GRP - 1),
                    )
                return h if gi == NG - 1 else None

            def emit_normalize(h):
                # normalize + elu -> h_local^T tile for this head
                psum_h, psum_d = psums.pop(h)
                rcp = work.tile([1, R], f32, tag="rcp", name=f"rcp{h}")
                nc.vector.reciprocal(out=rcp, in_=psum_d[0:1, :])
                rb_sb = work.tile([128, R], f32, tag="rb", name=f"rb{h}")
                nc.gpsimd.partition_broadcast(out_ap=rb_sb[:, :],
                                              in_ap=rcp[0:1, :])
                hn = work.tile([128, R], f32, tag="hn", name=f"hn{h}")
                nc.vector.tensor_tensor(out=hn, in0=psum_h, in1=rb_sb,
                                        op=mult)
                eh = work.tile([128, R], dt, tag="eh", name=f"eh{h}")
                nc.scalar.activation(out=eh, in_=hn, func=Exp)
                nc.vector._custom_dve(
                    ELU_SEL,
                    out=hloc_sb[:, h * R: (h + 1) * R],
                    in0=hn, in1=eh, s0=1.0, s1=0.0, imm2=0.0,
                )
                for rt in range(RT):
                    nc.tensor.matmul(
                        out=pWall[:, rt * F2p: (rt + 1) * F2p],
                        lhsT=hloc_sb[:, h * R + rt * 128: h * R + (rt + 1) * 128],
                        rhs=w2a_sb[:, h * F2p: (h + 1) * F2p],
                        start=(h == 0 and rt == 0),
                        stop=(h == HEADS - 1 and rt == RT - 1),
                        skip_group_check=True,
                    )

            # Normalize chains for heads 0..2 are deferred 2 slots so their
            # DVE ops don't sit between two heads' score production in the
            # in-order DVE queue (they would stall PE at each head boundary).
            pending_norm = []
            for s in range(len(slots) + PIPE):
                if s < len(slots):
                    issue_scores(s, slots)
                if s >= PIPE:
                    done = issue_matmuls(s - PIPE)
                    if done is not None:
                        if done == HEADS - 1:
                            emit_normalize(done)
                        else:
                            pending_norm.append((s + 2, done))
                for due, h in list(pending_norm):
                    if s >= due:
                        emit_normalize(h)
                        pending_norm.remove((due, h))

            if phases == "l1":
                for rt in range(RT):
                    nc.sync.dma_start(out=out_d[rt * 128:(rt + 1) * 128, :],
                                      in_=identf_sb[:, 0:F2])
                continue

            # ---- P5/P6: layer-2 projections + half-split gather ----------
            # gf_sb is t-major (t = c*RT + rt), PAY cols per tile
            gf_sb = const.tile([128, JT * PAY], dt)
            gf3 = gf_sb[:, :].rearrange("p (c q) -> p c q", c=cfg.CORES)
            gfF3 = gf_sb[:, :].bitcast(f32).rearrange(
                "p (c q) -> p c q", c=cfg.CORES)
            rq2r_sb = const.tile([128, JT], f32)
            rq2q_sb = const.tile([128, JT], f32)
            rq2r3 = rq2r_sb[:, :].rearrange("p (c r) -> p c r", c=cfg.CORES)
            rq2q3 = rq2q_sb[:, :].rearrange("p (c r) -> p c r", c=cfg.CORES)
            gs_sb = const.tile([128, RT * PAY], dt)
            g2row_sb = const.tile([1, R], dt)
            for rt in range(RT):
                pW = pWall[:, rt * F2p: (rt + 1) * F2p]
                b = rt * PAY
                nc.scalar.copy(out=gs_sb[:, b: b + F2], in_=pW[:, 0:F2])
                nc.vector.memset(gs_sb[:, b + F2: b + F2 + 2], 1.0)
                gsf = gs_sb[:, b + F2 + 2: b + PAY].bitcast(f32)
                nc.vector.tensor_copy(out=gsf, in_=pW[:, F2: F2 + 2])
                # local s1' row -> G2 row chunk
                pt2 = pss.tile([1, 128], f32, tag="sm", name=f"pt2_{rt}")
                nc.tensor.transpose(
                    out=pt2, in_=gsf[:, 0:1], identity=identf_sb,
                )
                nc.scalar.activation(
                    out=g2row_sb[0:1, rt * 128: (rt + 1) * 128], in_=pt2,
                    func=Exp, scale=BETA)
                if rt % 2 == 1:
                    gh = rt // 2
                    nc.sync.dma_start(
                        out=gsend[gh][:, :],
                        in_=gs_sb[:, (rt - 1) * PAY: (rt + 1) * PAY])
                    if collective:
                        nc.gpsimd.collective_compute(
                            "AllGather", bypass,
                            replica_groups=[list(range(cfg.CORES))],
                            ins=[gsend[gh].opt()], outs=[gfull[gh].opt()],
                        )
                    else:
                        nc.sync.dma_start(
                            out=gfull[gh].rearrange("(c p) q -> c p q", p=128),
                            in_=gsend[gh][:, :].unsqueeze(0).broadcast_to(
                                (cfg.CORES, 128, 2 * PAY)))
                    gff3 = gfull[gh].rearrange("(c p) q -> p c q", p=128)
                    nc.scalar.dma_start(
                        out=gf3[:, :, gh * 2 * PAY: (gh + 1) * 2 * PAY],
                        in_=gff3)
                    for ri in range(2):
                        rt2 = gh * 2 + ri
                        s2c = rt2 * (PAY // 2) + (F2 + 2) // 2 + 1
                        nc.scalar.activation(
                            out=rq2r3[:, :, rt2: rt2 + 1],
                            in_=gfF3[:, :, s2c: s2c + 1], func=Exp, scale=-BETA)
                        nc.scalar.activation(
                            out=rq2q3[:, :, rt2: rt2 + 1],
                            in_=gfF3[:, :, s2c: s2c + 1], func=Exp, scale=1.0)

            g2b = const.tile([128, R], dt)
            nc.gpsimd.partition_broadcast(
                out_ap=g2b[:, :], in_ap=g2row_sb[0:1, :])

            # ---- P8: layer-2 attention + aggregation ---------------------
            psum2 = ps2.tile([F2 + 1, R], f32, tag="big2", name="psum2")
            slots2 = [(rt, cq) for rt in range(RT) for cq in range(NCQ)]

            def issue_matmuls2(s):
                rt, cq = slots2[s]
                gi = rt * NCQ + cq
                tiles, yg = store.pop((True, s))
                for k, t in enumerate(tiles):
                    nc.tensor.matmul(
                        out=psum2,
                        lhsT=gf_sb[:, t * PAY: t * PAY + F2 + 1],
                        rhs=yg[:, k * R: (k + 1) * R],
                        start=(gi == 0 and k == 0),
                        stop=(gi == NG - 1 and k == GRP - 1),
                    )

            for s in range(len(slots2) + 2):
                if s < len(slots2):
                    issue_scores(s, slots2, g2=True)
                if s >= 2:
                    issue_matmuls2(s - 2)

            # ---- P9: finalize: transpose, normalize, store ---------------
            o2 = const.tile([F2 + 1, R], f32)
            nc.scalar.copy(out=o2, in_=psum2)
            ostage = const.tile([128, RT * F2], f32)
            for rt in range(RT):
                pT2 = pss.tile([128, F2 + 1], f32, tag="sm", name=f"pT2_{rt}")
                nc.tensor.transpose(
                    out=pT2,
                    in_=o2[:, rt * 128: (rt + 1) * 128],
                    identity=identf_sb[0: F2 + 1, 0: F2 + 1],
                )
                rc = work.tile([128, 1], f32, tag="rc", name=f"rc{rt}")
                nc.vector.reciprocal(out=rc, in_=pT2[:, F2: F2 + 1])
                nc.vector.tensor_scalar(
                    out=ostage[:, rt * F2: (rt + 1) * F2],
                    in0=pT2[:, 0:F2], scalar1=rc, scalar2=0.0,
                    op0=mult, op1=bypass,
                )
            nc.sync.dma_start(
                out=out_d.rearrange("(r p) f -> p r f", p=128),
                in_=ostage[:, :].rearrange("p (r f) -> p r f", r=RT))

    nc.compile()
    return nc


# --------------------------------------------------------------------------
# Host-side prep / sharding
# --------------------------------------------------------------------------

def host_prep(cfg: Cfg, g, inputs, W1, a1, W2, a2):
    N, C, H, HEADS, F2, R = cfg.N, cfg.C, cfg.H, cfg.HEADS, cfg.F2, cfg.R
    RT, CT = cfg.RT, cfg.CT
    X = np.asarray(inputs, np.float32)
    W1 = np.asarray(W1, np.float32)
    a1 = np.asarray(a1, np.float32)
    W2 = np.asarray(W2, np.float32)
    a2 = np.asarray(a2, np.float32)

    def tile128(A):
        # [k*128, cols] row-major -> partition-major [128, k*cols]
        k = A.shape[0] // 128
        return np.ascontiguousarray(
            A.reshape(k, 128, A.shape[1]).transpose(1, 0, 2).reshape(128, -1)
        )

    XT = np.ascontiguousarray(X.T).astype(BF16)                       # [C, N]
    w1c = tile128(np.ascontiguousarray(
        W1.transpose(1, 0, 2).reshape(C, HEADS * H)).astype(BF16))
    # fused score projections: [C, 8] interleaved (s1_h, s2_h)
    ws1_full = np.empty((C, 2 * HEADS), np.float32)
    for h in range(HEADS):
        ws1_full[:, 2 * h] = W1[h] @ a1[h, :H, 0]
        ws1_full[:, 2 * h + 1] = W1[h] @ a1[h, H:, 0]
    ws1 = tile128(ws1_full.astype(BF16))
    w2_full = np.concatenate(
        [W2, W2 @ a2[:F2], W2 @ a2[F2:]], axis=1)                     # [HH, 66]
    w2a = tile128(w2_full.astype(BF16))
    ident = np.eye(128, dtype=BF16)
    identf = np.eye(128, dtype=np.float32)

    adj = np.asarray(g) > 0
    in_maps = []
    for c in range(cfg.CORES):
        rows = slice(c * R, (c + 1) * R)
        mb = adj[rows].T.astype(BF16)                                 # [N, R]
        mbt = tile128(np.ascontiguousarray(mb)).reshape(128, cfg.JT, R)
        # reorder tiles rt-major: slot (rt*CORES + cc) <- tile cc*RT + rt
        idx = [cc * RT + rt for rt in range(RT) for cc in range(cfg.CORES)]
        mbt = np.ascontiguousarray(mbt[:, idx, :]).reshape(128, -1)
        # xtl rt-major: [c%128, rt, ct, j]
        xl = np.asarray(XT[:, rows])                                  # [C, R]
        xl = xl.reshape(CT, 128, RT, 128).transpose(1, 2, 0, 3).reshape(128, -1)
        in_maps.append({
            "xtloc": np.ascontiguousarray(xl),
            "mb": mbt,
            "w1c": w1c, "ws1": ws1, "w2a": w2a,
            "ident": ident, "identf": identf,
        })
    return in_maps


_NC_CACHE = {}


def get_compiled(cfg: Cfg):
    nc = _NC_CACHE.get(cfg)
    if nc is None:
        nc = build_gat_nc(cfg)
        _NC_CACHE[cfg] = nc
    return nc


def kernel(g, inputs, W1, a1, W2, a2):
    cfg = FULL
    nc = get_compiled(cfg)
    in_maps = host_prep(cfg, g, inputs, W1, a1, W2, a2)
    res = run_bass_kernel_spmd(nc, in_maps, core_ids=list(range(cfg.CORES)))
    out = np.concatenate(
        [np.asarray(res.results[c]["out"], np.float32) for c in range(cfg.CORES)],
        axis=0,
    )
    return out
